# revision 10
# baseline (speedup 1.0000x reference)
"""nn_CRF kernel for 8 Trainium2 NeuronCores.

kernel(**inputs) takes the FULL inputs (x [64,1024,512], W [512,24], b [24],
trans [24,24], left_boundary [24], right_boundary [24]) and returns the full
reference outputs (decoded [64,1024] int32, pot [64,1024,24] f32,
lens [64] int32, trans [24,24] f32).

Sharding: data-parallel over batch B=64 -> 8 sequences per core; W/trans/
boundaries (as precomputed layout constants) replicated to every core.

Per-core device program (_build_nc):
  1. Projection pot = x @ W + b (+boundary rows) on PE, streamed from HBM in
     128-token tiles (DMA-bound). pot -> HBM output; drift-centered transpose
     -> SBUF "potBig" laid out for the scan.
  2. Viterbi forward (alpha) and backward (gamma) max-scans.
     gamma_t[j] = max_j'(gamma_{t+1}[j'] + trans[j,j']) + pot[t,j], so
     s_t[j] = alpha_t[j] + gamma_t[j] - pot[t,j] is the best score of a path
     through (t,j); argmax_j decodes Viterbi without backtrack. The serial
     T-dependence is chunked: chunk 0 fwd / chunk C-1 bwd run exactly, the
     rest in one wide "mega" chain per direction with Delta-step warmups
     (scans coalesce to a per-sequence constant shift, invisible to argmax).
     Per scan step: DVE diag-place state -> PE matmul (block-ones = state
     broadcast + trans rows) accumulated on PE matmul of pot columns -> DVE
     grouped max. Chunk-boundary columns go to the host, which checks
     coalescence and exactly re-decodes any failing sequence.
  3. Decode sweep: s -> per-t argmax (lowest-index ties like jnp.argmax) and
     top1-top2 margins; rows with margins below GAP_EPS are re-decoded on the
     host from the device potentials (exact backpointer Viterbi).
"""

from contextlib import ExitStack

import numpy as np

import concourse.bacc as bacc
import concourse.bass as bass
import concourse.mybir as mybir
from concourse.bass import ds
from concourse.tile import TileContext

F32 = mybir.dt.float32
N = 24
B4 = 4
P96 = B4 * N
PK = P96 + N

B, T, D = 64, 1024, 512
NCORES = 8
BL = B // NCORES
C, DELTA, TT = 8, 24, 128
GAP_EPS = 3e-4       # host re-decodes rows with any decode margin below this
CO_EPS = 2e-3        # coalescence spread threshold


# ---------------------------------------------------------------- host consts
def _host_consts(W, bvec, trans, left_b, right_b, drift,
                 T=T, D=D, C=C, Delta=DELTA, TT=TT):
    K4 = D // 128
    c = {}
    c["W_sb"] = np.ascontiguousarray(
        W.reshape(K4, 128, N).transpose(1, 0, 2).reshape(128, K4 * N)
    ).astype(np.float32)
    c["identity"] = np.eye(128, dtype=np.float32)
    blockS = np.kron(np.eye(B4, dtype=np.float32), np.ones((N, N), np.float32))
    c["SXf"] = np.concatenate([blockS, np.tile(trans, (1, B4))], 0).astype(np.float32)
    c["SXb"] = np.concatenate([blockS, np.tile(trans.T, (1, B4))], 0).astype(np.float32)
    c["I96"] = np.eye(P96, dtype=np.float32)
    c["diagMask"] = np.tile(np.eye(N, dtype=np.float32), (B4, 1)).astype(np.float32)
    dm1 = np.eye(N, dtype=np.float32)
    c["dm_mega"] = np.tile(dm1, (1, 2 * max(C - 1, 1))).astype(np.float32)
    c["dm_ser"] = np.tile(dm1, (1, 2)).astype(np.float32)
    c["driftNegColT"] = np.full((TT, 1), -drift, np.float32)
    bvec = np.asarray(bvec, np.float32)
    lb = np.asarray(left_b, np.float32)
    rb = np.asarray(right_b, np.float32)
    ntt = T // TT
    bR = np.tile(bvec[None, :], (TT, B4))
    bF = bR.copy(); bF[0] += np.tile(lb, B4)
    if ntt == 1:
        bF[TT - 1] += np.tile(rb, B4)
    bL = bR.copy(); bL[TT - 1] += np.tile(rb, B4)
    c["bRep"] = bR; c["bRepF"] = bF; c["bRepL"] = bL
    c["iotaC"] = np.tile(np.arange(N, dtype=np.float32)[None, :], (128, B4))
    c["bigC"] = np.full((128, P96), 1e30, np.float32)
    c["negbigC"] = np.full((128, P96), -1e30, np.float32)
    return {k: np.ascontiguousarray(v, np.float32) for k, v in c.items()}


def _const_shapes(T=T, D=D, C=C, Delta=DELTA, TT=TT):
    K4 = D // 128
    return dict(
        W_sb=(128, K4 * N), identity=(128, 128), SXf=(PK, P96), SXb=(PK, P96),
        I96=(P96, P96), diagMask=(P96, N),
        dm_mega=(N, 48 * max(C - 1, 1)), dm_ser=(N, 48),
        driftNegColT=(TT, 1), bRep=(TT, P96), bRepF=(TT, P96), bRepL=(TT, P96),
        iotaC=(128, P96), bigC=(128, P96), negbigC=(128, P96),
    )


# ---------------------------------------------------------------- device code
def _build_nc(T=T, D=D, C=C, Delta=DELTA, TT=TT):
    BL_ = 8
    assert D % 128 == 0 and T % TT == 0 and T % C == 0
    CH = T // C
    assert CH > Delta >= 1
    NC = 2 * C
    NTL = CH + 2 * Delta + 1
    K4 = D // 128

    nc = bacc.Bacc("TRN2", target_bir_lowering=False)
    x_in = nc.declare_dram_parameter("x", [BL_, T, D], F32, isOutput=False)
    shapes = _const_shapes(T, D, C, Delta, TT)
    consts = {
        name: nc.declare_dram_parameter(name, list(shp), F32, isOutput=False)
        for name, shp in shapes.items()
    }
    pot_out = nc.declare_dram_parameter("pot", [BL_, T, N], F32, isOutput=True)
    tags_out = nc.declare_dram_parameter("tags", [T, BL_], F32, isOutput=True)
    gap_out = nc.declare_dram_parameter("gap", [T, BL_], F32, isOutput=True)
    bound_out = nc.declare_dram_parameter("bound", [P96, 4 * NC], F32, isOutput=True)

    with TileContext(nc) as tc, ExitStack() as ctx:
        cpool = ctx.enter_context(tc.tile_pool(name="consts", bufs=1))
        big = ctx.enter_context(tc.tile_pool(name="big", bufs=1))
        xpool = ctx.enter_context(tc.tile_pool(name="xp", bufs=8))
        xtp = ctx.enter_context(tc.tile_pool(name="xtp", bufs=4))
        pots = ctx.enter_context(tc.tile_pool(name="pots", bufs=3))
        sw = ctx.enter_context(tc.tile_pool(name="sw", bufs=3))
        ps_x = ctx.enter_context(tc.tile_pool(name="ps_x", bufs=1, space="PSUM"))
        ps_pp = ctx.enter_context(tc.tile_pool(name="ps_pp", bufs=1, space="PSUM"))
        ps_ser = ctx.enter_context(tc.tile_pool(name="ps_ser", bufs=2, space="PSUM"))
        ps_mf = ctx.enter_context(tc.tile_pool(name="ps_mf", bufs=2, space="PSUM"))
        ps_mb = ctx.enter_context(tc.tile_pool(name="ps_mb", bufs=2, space="PSUM"))

        ct = {}
        for name in shapes:
            t = cpool.tile(list(shapes[name]), F32, tag=name)
            nc.sync.dma_start(t[:], consts[name][:])
            ct[name] = t

        # PE "const warm" dummies: transposes carry at most ONE sync wait in
        # the ISA, so make PE observe every const-DMA queue up front. The
        # first reads only identity; later ones read one new const each. All
        # write the same psum tile so WAW keeps them in program order.
        dummy = ps_pp.tile([128, 128], F32, tag="ppot")
        nc.tensor.transpose(dummy[:], ct["identity"][:], ct["identity"][:])
        nc.tensor.transpose(
            dummy[ds(0, K4 * N), :], ct["W_sb"][:, :], ct["identity"][:]
        )
        nc.tensor.transpose(
            dummy[ds(0, P96), ds(0, PK)], ct["SXf"][:], ct["identity"][0:PK, 0:PK]
        )
        nc.tensor.transpose(
            dummy[ds(0, P96), ds(0, PK)], ct["SXb"][:], ct["identity"][0:PK, 0:PK]
        )
        nc.tensor.transpose(
            dummy[ds(0, P96), ds(0, P96)], ct["I96"][:], ct["identity"][0:P96, 0:P96]
        )

        # DVE/ACT const warm: observe each const's DMA queue once up front so
        # steady-state DVE/ACT instructions carry few sync waits.
        warm = big.tile([1, 8], F32, tag="warm")
        for nm in ("bRep", "bRepF", "bRepL", "diagMask", "dm_mega", "dm_ser",
                   "iotaC", "bigC", "negbigC"):
            nc.vector.tensor_copy(warm[0:1, 0:1], ct[nm][0:1, 0:1])
        nc.scalar.copy(warm[0:1, 1:2], ct["driftNegColT"][0:1, 0:1])

        alphaBig = big.tile([P96, NTL * NC], F32, tag="alphaBig")
        gammaBig = big.tile([P96, NTL * NC], F32, tag="gammaBig")
        potBig = big.tile([P96, NTL * NC], F32, tag="potBig")
        nc.vector.memset(potBig[:], 0.0)

        a2 = {}
        for nm, dmc, FF in (
            ("mf", "dm_mega", 48 * max(C - 1, 1)),
            ("mb", "dm_mega", 48 * max(C - 1, 1)),
            ("sf", "dm_ser", 48),
            ("sb", "dm_ser", 48),
        ):
            t = big.tile([PK, FF], F32, tag="A2" + nm)
            nc.vector.tensor_copy(t[ds(P96, N), :], ct[dmc][:, 0:FF])
            a2[nm] = t

        # ================= projection =================
        # per (token-tile, b2): project the 4 b4-sequences into one [TT, 96]
        # tile, one PE transpose -> [96, TT], DVE copies into potBig columns.
        ntt = T // TT
        tt_order = []
        lo, hi = 0, ntt - 1
        while lo <= hi:
            tt_order.append(lo)
            if hi != lo:
                tt_order.append(hi)
            lo += 1
            hi -= 1

        for tt in tt_order:
            for b2 in range(2):
                ppot_t = ps_pp.tile([128, 128], F32, tag="ppot")
                ppot4 = ppot_t[ds(0, TT), ds(0, P96)]
                for b4 in range(B4):
                    b = b4 * 2 + b2
                    xt = xpool.tile([128, D], F32, tag="xt")
                    nc.sync.dma_start(xt[:], x_in[b, ds(tt * TT, TT), :])
                    for k in range(K4):
                        pxt = ps_x.tile([128, 128], F32, tag="psx")
                        nc.tensor.transpose(
                            pxt[:], xt[:, ds(k * 128, 128)], ct["identity"][:]
                        )
                        xT = xtp.tile([128, 128], F32, tag="xT")
                        nc.scalar.copy(xT[:], pxt[:])
                        nc.tensor.matmul(
                            ppot_t[ds(0, TT), ds(b4 * N, N)],
                            xT[:], ct["W_sb"][:, ds(k * N, N)],
                            start=(k == 0), stop=(k == K4 - 1),
                        )
                brep = "bRep"
                if tt == 0:
                    brep = "bRepF"
                elif tt == ntt - 1:
                    brep = "bRepL"
                pot_sb = pots.tile([TT, P96], F32, tag="pot_sb")
                nc.scalar.copy(pot_sb[:], ppot4)
                psb4 = pots.tile([TT, P96], F32, tag="psb")
                nc.vector.tensor_add(psb4[:], pot_sb[:], ct[brep][:])
                pot_v = pot_out[:, :, :].rearrange("(b4 b2) t n -> b2 t b4 n", b2=2)
                nc.sync.dma_start(
                    pot_v[b2, ds(tt * TT, TT), :, :],
                    psb4[:, :].rearrange("t (b4 n) -> t b4 n", n=N),
                )
                # drift-centered copy via ACT (bias = -drift per partition)
                pc4a = pots.tile([TT, P96], F32, tag="pc")
                nc.scalar.activation(
                    pc4a[:], psb4[:], mybir.ActivationFunctionType.Identity,
                    bias=ct["driftNegColT"][:],
                )
                ptr_t = ps_pp.tile([128, 128], F32, tag="ppot")
                ptr = ptr_t[ds(0, P96), ds(0, TT)]
                # regular matmul (not transpose-mode): LDW+MM can carry two
                # sync waits, transpose-mode only one
                nc.tensor.matmul(
                    ptr, pc4a[:], ct["identity"][0:TT, 0:TT],
                    start=True, stop=True,
                )
                pts = pots.tile([P96, TT], F32, tag="pts")
                nc.scalar.copy(pts[:], ptr)

                def pot_copy(tau0, ln, dst_tloc, dst_c):
                    dst = potBig[:, :].rearrange("p (q t) -> p q t", t=NTL)[
                        :, dst_c * 2 + b2, ds(dst_tloc + Delta, ln)
                    ]
                    nc.vector.tensor_copy(dst, pts[:, ds(tau0, ln)])

                t0 = tt * TT
                c_lo, c_hi = t0 // CH, (t0 + TT - 1) // CH
                for cc in range(c_lo, c_hi + 1):
                    s = max(t0, cc * CH)
                    e = min(t0 + TT, (cc + 1) * CH)
                    if e > s:
                        pot_copy(s - t0, e - s, s - cc * CH, cc)
                for cn in range(c_lo, min(c_hi + 2, C)):
                    s = max(t0, cn * CH - Delta)
                    e = min(t0 + TT, cn * CH)
                    if e > s and cn >= 1:
                        pot_copy(s - t0, e - s, s - cn * CH, cn)
                for cp in range(max(c_lo - 1, 0), c_hi + 1):
                    s = max(t0, (cp + 1) * CH)
                    e = min(t0 + TT, (cp + 1) * CH + Delta + 1)
                    if e > s and cp <= C - 2:
                        pot_copy(s - t0, e - s, s - cp * CH, cp)

        # ================= scan chains =================
        def cols_ap(bigt, tloc, chain):
            v = bigt[:, :].rearrange("p (q t) -> p q t", t=NTL)
            return v[:, ds(chain[0] * 2, len(chain) * 2), tloc + Delta]

        def cols_ap_bcast(bigt, tloc, chain):
            a = cols_ap(bigt, tloc, chain)
            return a.unsqueeze(2).broadcast_to((P96, len(chain) * 2, N))

        def emit_step(bigt, tloc, pot_tloc, chain, a2t, pst, SX, tag):
            L = len(chain)
            src = cols_ap_bcast(bigt, tloc, chain)
            dm = ct["diagMask"][:, :].unsqueeze(1).broadcast_to((P96, 2 * L, N))
            nc.vector.tensor_mul(
                a2t[ds(0, P96), :].rearrange("p (g i) -> p g i", i=N), dm, src
            )
            ps = pst.tile([P96, 48 * L], F32, tag=tag)
            nc.tensor.matmul(
                ps[:].rearrange("p (g i) -> p g i", i=N),
                ct["I96"][:],
                cols_ap_bcast(potBig, pot_tloc, chain),
                start=True, stop=False,
            )
            nc.tensor.matmul(ps[:], SX[:], a2t[:], start=False, stop=True)
            return ps

        def emit_reduce(bigt, tloc, chain, ps):
            nc.vector.tensor_reduce(
                cols_ap(bigt, tloc, chain),
                ps[:].rearrange("p (g i) -> p g i", i=N),
                mybir.AxisListType.X,
                mybir.AluOpType.max,
            )

        mega = list(range(1, C))
        megab = list(range(0, C - 1))
        nc.vector.tensor_copy(cols_ap(alphaBig, 0, [0]), cols_ap(potBig, 0, [0]))
        nc.vector.tensor_copy(
            cols_ap(gammaBig, CH - 1, [C - 1]), cols_ap(potBig, CH - 1, [C - 1])
        )
        if C > 1:
            nc.vector.tensor_copy(
                cols_ap(alphaBig, -Delta, mega), cols_ap(potBig, -Delta, mega)
            )
            nc.vector.tensor_copy(
                cols_ap(gammaBig, CH + Delta - 1, megab),
                cols_ap(potBig, CH + Delta - 1, megab),
            )

        for step in range(CH + Delta + 1):
            tl = 1 + step
            if tl <= CH:
                ps = emit_step(alphaBig, tl - 1, tl, [0], a2["sf"], ps_ser, ct["SXf"], "ser")
                emit_reduce(alphaBig, tl, [0], ps)
            tl = CH - 2 - step
            if tl >= -1:
                ps = emit_step(gammaBig, tl + 1, tl, [C - 1], a2["sb"], ps_ser, ct["SXb"], "ser")
                emit_reduce(gammaBig, tl, [C - 1], ps)
            if C > 1:
                tl = -Delta + 1 + step
                if tl <= CH:
                    ps = emit_step(alphaBig, tl - 1, tl, mega, a2["mf"], ps_mf, ct["SXf"], "mf")
                    emit_reduce(alphaBig, tl, mega, ps)
                tl = CH + Delta - 2 - step
                if tl >= -1:
                    ps = emit_step(gammaBig, tl + 1, tl, megab, a2["mb"], ps_mb, ct["SXb"], "mb")
                    emit_reduce(gammaBig, tl, megab, ps)

        allc = list(range(C))
        for i, (bigt, tl) in enumerate(
            ((alphaBig, 0), (alphaBig, CH), (gammaBig, CH - 1), (gammaBig, -1))
        ):
            nc.sync.dma_start(bound_out[:, ds(i * NC, NC)], cols_ap(bigt, tl, allc))

        # ================= sweep =================
        for c in range(C):
            for b2 in range(2):

                def rcols(bigt):
                    v = bigt[:, :].rearrange("p (q t) -> p q t", t=NTL)
                    return v[:, c * 2 + b2, ds(Delta, CH)]

                st = sw.tile([P96, CH], F32, tag="s")
                nc.vector.tensor_add(st[:], rcols(alphaBig), rcols(gammaBig))
                nc.vector.tensor_sub(st[:], st[:], rcols(potBig))
                pss_t = ps_ser.tile([CH, P96], F32, tag="ser")
                pss = pss_t[:, :]
                nc.tensor.transpose(pss, st[:], ct["identity"][0:P96, 0:P96])
                m1 = sw.tile([CH, B4], F32, tag="m1")
                nc.vector.tensor_reduce(
                    m1[:], pss.rearrange("t (b j) -> t b j", j=N),
                    mybir.AxisListType.X, mybir.AluOpType.max,
                )
                m1b = m1[:, :].unsqueeze(2).broadcast_to((CH, B4, N))
                eqm = sw.tile([CH, P96], mybir.dt.uint8, tag="eqm")
                nc.vector.tensor_tensor(
                    eqm[:].rearrange("t (b j) -> t b j", j=N),
                    pss.rearrange("t (b j) -> t b j", j=N),
                    m1b, mybir.AluOpType.is_equal,
                )
                sel = sw.tile([CH, P96], F32, tag="sel")
                nc.vector.tensor_copy(sel[:], ct["bigC"][0:CH, :])
                nc.vector.copy_predicated(sel[:], eqm[:], ct["iotaC"][0:CH, :])
                tg = sw.tile([CH, B4], F32, tag="tg")
                nc.vector.tensor_reduce(
                    tg[:], sel[:].rearrange("t (b j) -> t b j", j=N),
                    mybir.AxisListType.X, mybir.AluOpType.min,
                )
                sel2 = sw.tile([CH, P96], F32, tag="sel2")
                nc.vector.tensor_copy(sel2[:], pss)
                nc.vector.copy_predicated(sel2[:], eqm[:], ct["negbigC"][0:CH, :])
                m2 = sw.tile([CH, B4], F32, tag="m2")
                nc.vector.tensor_reduce(
                    m2[:], sel2[:].rearrange("t (b j) -> t b j", j=N),
                    mybir.AxisListType.X, mybir.AluOpType.max,
                )
                gp = sw.tile([CH, B4], F32, tag="gp")
                nc.vector.tensor_sub(gp[:], m1[:], m2[:])
                tga = tags_out[ds(c * CH, CH), :].rearrange(
                    "t (b4 b2) -> t b4 b2", b2=2
                )[:, :, b2]
                gpa = gap_out[ds(c * CH, CH), :].rearrange(
                    "t (b4 b2) -> t b4 b2", b2=2
                )[:, :, b2]
                nc.sync.dma_start(tga, tg[:])
                nc.sync.dma_start(gpa, gp[:])

    return nc


_NC_CACHE = None


def _get_nc():
    global _NC_CACHE
    if _NC_CACHE is None:
        _NC_CACHE = _build_nc()
        _NC_CACHE.finalize()   # Bacc.compile(): wait legalization, reg alloc
    return _NC_CACHE


# ---------------------------------------------------------------- host repair
def _exact_decode_rows(pot, trans):
    """Reference-order fp32 Viterbi (backpointers + backtrack) for rows of pot."""
    R, T_, N_ = pot.shape
    a = pot[:, 0].copy()
    bps = np.zeros((T_ - 1, R, N_), np.int64)
    for t in range(1, T_):
        sc = a[:, :, None] + trans[None]
        bps[t - 1] = sc.argmax(axis=1)
        a = sc.max(axis=1).astype(np.float32) + pot[:, t]
    tags = np.zeros((R, T_), np.int32)
    tags[:, T_ - 1] = a.argmax(axis=1)
    rr = np.arange(R)
    for t in range(T_ - 2, -1, -1):
        tags[:, t] = bps[t][rr, tags[:, t + 1]]
    return tags


# ---------------------------------------------------------------- entry point
def kernel(x, W, b, trans, left_boundary, right_boundary):
    from concourse.bass_utils import run_bass_kernel_spmd

    x = np.ascontiguousarray(np.asarray(x, np.float32))
    W = np.asarray(W, np.float32)
    bvec = np.asarray(b, np.float32)
    trans = np.ascontiguousarray(np.asarray(trans, np.float32))
    lb = np.asarray(left_boundary, np.float32)
    rb = np.asarray(right_boundary, np.float32)

    # drift: rough per-step best-path gain, from one sequence's potentials
    drift = float(np.mean((x[0].astype(np.float32) @ W + bvec).max(axis=1)))
    consts = _host_consts(W, bvec, trans, lb, rb, drift)

    nc = _get_nc()
    in_maps = [
        {"x": x[ci * BL : (ci + 1) * BL], **consts} for ci in range(NCORES)
    ]
    res = run_bass_kernel_spmd(nc, in_maps, core_ids=list(range(NCORES)))

    pot = np.empty((B, T, N), np.float32)
    decoded = np.empty((B, T), np.int32)
    NC = 2 * C
    for ci in range(NCORES):
        r = res.results[ci]
        pot[ci * BL : (ci + 1) * BL] = r["pot"]
        tags = r["tags"].astype(np.int32).T      # [T,BL] -> [BL,T]
        gap = r["gap"].T                          # [BL, T]
        bound = r["bound"]
        bad = np.zeros(BL, bool)
        bad |= (gap < GAP_EPS).any(axis=1)
        A0, ACH = bound[:, 0:NC], bound[:, NC : 2 * NC]
        GC, GM1 = bound[:, 2 * NC : 3 * NC], bound[:, 3 * NC : 4 * NC]
        for b4 in range(B4):
            blk = slice(b4 * N, b4 * N + N)
            for b2 in range(2):
                bl = b4 * 2 + b2
                for c in range(C - 1):
                    d = ACH[blk, c * 2 + b2] - A0[blk, (c + 1) * 2 + b2]
                    if d.max() - d.min() > CO_EPS:
                        bad[bl] = True
                for c in range(1, C):
                    d = GM1[blk, c * 2 + b2] - GC[blk, (c - 1) * 2 + b2]
                    if d.max() - d.min() > CO_EPS:
                        bad[bl] = True
        if bad.any():
            rows = np.where(bad)[0]
            fixed = _exact_decode_rows(
                pot[ci * BL : (ci + 1) * BL][rows], trans
            )
            tags[rows] = fixed
        decoded[ci * BL : (ci + 1) * BL] = tags

    lens = np.full((B,), T, np.int32)
    return decoded, pot, lens, trans
